# revision 9
# baseline (speedup 1.0000x reference)
"""Trainium2 kernel for nn_Gate (dense_mlp, memory-bound).

Computation: global avg-pool over x[16,512,128,128] (512MB — the entire
cost), then a tiny binarized 2-layer MLP tail ([16,512] matrices,
512x512 sign-weights, BatchNorm over batch, PReLU).

Strategy: data-parallel over batch — 2 samples/core on 8 cores. Each
core streams its 64MB shard through SBUF and reduces over H*W on the
vector engine, emitting per-(sample,channel) sums [128, 8]. The
microscopic tail (~4M MACs on [16,512]) runs on host in numpy.
"""

import numpy as np

_B, _C, _H, _W = 16, 512, 128, 128
_HW = _H * _W            # 16384 elements pooled per (b, c)
_NCORES = 8
_BPC = _B // _NCORES     # samples per core
_ROWS = _BPC * _C        # 1024 rows of length HW per core
_NBLK = _ROWS // 128     # 8 partition blocks per core
_BN_EPS = 1e-5

_CACHE = {}


def _bench_reps(nreps):
    # benchmark-only: repeat the block loop so (t[K]-t[1])/(K-1) isolates
    # device exec time from axon dispatch overhead
    for _ in range(nreps):
        yield from range(_NBLK)


def _build(nreps=1):
    import concourse.bacc as bacc
    import concourse.tile as tile
    from concourse import mybir

    # Bacc (not raw Bass): its compile() pipeline runs
    # generate_event_semaphores, which splits multi-wait instructions to
    # satisfy TRN2's 1-wait-per-instruction constraint.
    nc = bacc.Bacc(None, target_bir_lowering=False)
    x_in = nc.declare_dram_parameter("x", [_ROWS, _HW], mybir.dt.float32, isOutput=False)
    out = nc.declare_dram_parameter("out", [128, _NBLK * 128], mybir.dt.float32, isOutput=True)

    with tile.TileContext(nc) as tc:
        with (
            tc.tile_pool(name="work", bufs=2) as work_pool,
            tc.tile_pool(name="stage", bufs=2) as stage_pool,
        ):
            for i in _bench_reps(nreps):
                t = work_pool.tile([128, _HW], mybir.dt.float32)
                nc.sync.dma_start(out=t[:], in_=x_in[i * 128:(i + 1) * 128, :])
                partial = stage_pool.tile([128, 128], mybir.dt.float32)
                # first-stage reduction (16384 -> 128); host sums the
                # remaining 128 per row. Keeps each out-DMA dependent on
                # exactly one reduce (compiler caps sync waits per DMA).
                nc.vector.tensor_reduce(
                    out=partial[:],
                    in_=t[:].rearrange("p (a b) -> p a b", b=128),
                    axis=mybir.AxisListType.X,
                    op=mybir.AluOpType.add,
                )
                nc.sync.dma_start(
                    out=out[:, i * 128:(i + 1) * 128], in_=partial[:]
                )
    nc.finalize()  # Bacc: runs alloc_regs + generate_event_semaphores
    return nc


def _pool_on_device(x, trace=False):
    """x: [16,512,128,128] f32 -> per-(b,c) means [16,512] f32."""
    from concourse.bass_utils import run_bass_kernel_spmd

    if "nc" not in _CACHE:
        _CACHE["nc"] = _build()
    nc = _CACHE["nc"]

    xf = np.ascontiguousarray(x.reshape(_B, _C * _HW))
    in_maps = [
        {"x": xf[c * _BPC:(c + 1) * _BPC].reshape(_ROWS, _HW)}
        for c in range(_NCORES)
    ]
    res = run_bass_kernel_spmd(nc, in_maps, core_ids=list(range(_NCORES)), trace=trace)
    _CACHE["last_exec_ns"] = res.exec_time_ns
    # out[p, i*128 + k] holds partial sum k of row (i*128 + p)
    sums = np.concatenate(
        [
            res.results[c]["out"].reshape(128, _NBLK, 128).sum(axis=-1)
            .T.reshape(_BPC, _C)
            for c in range(_NCORES)
        ],
        axis=0,
    )
    return sums * np.float32(1.0 / _HW)


def _tail(p, move1_bias, w1, bn_gamma, bn_beta, lbias, prelu_a, move2_bias, w2):
    """Binarized MLP tail on [16,512] pooled features.

    Runs as eager jax, mirroring the reference op-for-op: BatchNorm's
    (h - mu) hits exact ties (h equals the batch mean in exact
    arithmetic), where sign() is decided by rounding dust — only the
    identical op sequence reproduces the same dust. p itself only enters
    through sign(p + move1_bias), so the device pooling merely has to
    get signs right.
    """
    import jax
    import jax.numpy as jnp

    sg = jax.lax.stop_gradient

    def approx_sign(x):
        out3 = jnp.where(x < -1.0, -1.0,
               jnp.where(x < 0.0, x * x + 2.0 * x,
               jnp.where(x < 1.0, -x * x + 2.0 * x, 1.0)))
        return sg(jnp.sign(x) - out3) + out3

    def react_linear(x, move_bias, w):
        xb = approx_sign(x + move_bias)
        scale = sg(jnp.mean(jnp.abs(w), axis=1, keepdims=True))
        bw_ng = scale * jnp.sign(w)
        cw = jnp.clip(w, -1.0, 1.0)
        bw = sg(bw_ng - cw) + cw
        return xb @ bw.T

    h = react_linear(jnp.asarray(p), jnp.asarray(move1_bias), jnp.asarray(w1))
    mu = jnp.mean(h, axis=0)
    var = jnp.var(h, axis=0)
    h = (h - mu) * jax.lax.rsqrt(var + _BN_EPS) * jnp.asarray(bn_gamma) + jnp.asarray(bn_beta)
    h = h + jnp.asarray(lbias)
    h = jnp.where(h >= 0, h, jnp.asarray(prelu_a) * h)
    beta = react_linear(h, jnp.asarray(move2_bias), jnp.asarray(w2))
    return np.asarray(beta[:, :, None, None], dtype=np.float32)


def kernel(x, move1_bias, w1, bn_gamma, bn_beta, lbias, prelu_a, move2_bias, w2,
           _trace=False):
    x = np.asarray(x, dtype=np.float32)
    p = _pool_on_device(x, trace=_trace)
    return _tail(
        p,
        np.asarray(move1_bias, np.float32),
        np.asarray(w1, np.float32),
        np.asarray(bn_gamma, np.float32),
        np.asarray(bn_beta, np.float32),
        np.asarray(lbias, np.float32),
        np.asarray(prelu_a, np.float32),
        np.asarray(move2_bias, np.float32),
        np.asarray(w2, np.float32),
    )
